# revision 14
# baseline (speedup 1.0000x reference)
"""Trainium2 Bass kernel for nn_Cross_Attn_Token_to_Image (dense transformer cross-attention).

Sharding (8 cores, no collectives): core = (batch b in {0,1}) x (head-group g in {0..3},
2 heads each).  Host pre-transposes/casts activations to fp16 [E, tokens] slices and
pre-slices weights column-wise (q/cond/k1/k2/v) / row-wise (out_proj).  Each core
computes a partial [2048, 1024] output (its heads' contribution through its Wo row
block); the host sums the 4 partials per batch and folds the linear bias terms
(bv @ Wo + bo, exact because attention rows sum to 1).

Device dataflow per core (all matmuls fp16 operands, fp32 PSUM accumulation), software
pipelined by head: for each of the 2 heads, project that head's k1hT/k2hT [d,nk],
vh [nk,d], qpT/cpT [d,nq] (PE-dense), then run its 16 nq-tile attention iterations
(ACT-dense: exp with accum_out row-sums) while the PE projects the next head's data.
Per iteration: s = qhT^T k1hT on PE; p = exp(s*scale) on ACT; rows normalized by
beta = 0.5*C/r via DVE in-place scale; p1+p2 combined on DVE; transposed on PE
(fp16 PSUM); uT = vh^T pT accumulated on PE; final out = uT^T @ Wo row-block scaled
by 1/C, emitted per tile once both heads' uT exist.
"""
import numpy as np

NQ = 2048
NK = 4096
E = 1024
D = 128
CNORM = 16384.0
SCALE = float(1.0 / np.sqrt(D))

_CACHED_NC = None


def _build():
    from contextlib import ExitStack

    import concourse.tile as tile
    from concourse import bacc, mybir
    from concourse.masks import make_identity

    F16 = mybir.dt.float16
    F32 = mybir.dt.float32
    AX = mybir.AxisListType
    ALU = mybir.AluOpType
    ACTF = mybir.ActivationFunctionType

    nc = bacc.Bacc("TRN2", target_bir_lowering=False, debug=False, num_devices=8)

    qT = nc.dram_tensor("qt", [E, NQ], F16, kind="ExternalInput").ap()
    cT = nc.dram_tensor("ct", [E, NQ], F16, kind="ExternalInput").ap()
    kT = nc.dram_tensor("kt", [E, NK], F16, kind="ExternalInput").ap()
    vT = nc.dram_tensor("vt", [E, NK], F16, kind="ExternalInput").ap()
    w_dram = {
        w: nc.dram_tensor(w, [E, 256], F16, kind="ExternalInput").ap()
        for w in ("wq", "wc", "wk1", "wk2", "wv")
    }
    wo = nc.dram_tensor("wo", [256, E], F16, kind="ExternalInput").ap()
    b_dram = {
        b: nc.dram_tensor(b, [128, 2], F32, kind="ExternalInput").ap()
        for b in ("bq", "bc", "bk1", "bk2")
    }
    outp = nc.dram_tensor("outp", [NQ, E], F32, kind="ExternalOutput").ap()

    with tile.TileContext(nc) as tc, ExitStack() as ctx:
        const = ctx.enter_context(tc.tile_pool(name="const", bufs=1))
        persist = ctx.enter_context(tc.tile_pool(name="persist", bufs=1))

        b_sb = {}
        for b in ("bq", "bc", "bk1", "bk2"):
            b_sb[b] = const.tile([128, 2], F32, name=b)
            nc.sync.dma_start(out=b_sb[b], in_=b_dram[b])
        identity16 = const.tile([128, 128], F16)
        make_identity(nc, identity16)
        wo_sb = const.tile([128, 2, E], F16)
        nc.sync.dma_start(out=wo_sb, in_=wo.rearrange("(c p) n -> p c n", p=128))

        w_sb = {}
        for w in ("wq", "wc", "wk1", "wk2", "wv"):
            w_sb[w] = const.tile([128, 8, 256], F16, name=w)
            nc.sync.dma_start(
                out=w_sb[w], in_=w_dram[w].rearrange("(c p) n -> p c n", p=128)
            )

        # double-buffered by head: projections for head h land in slot h
        qpT = persist.tile([128, 2, NQ], F16, name="qpT")
        cpT = persist.tile([128, 2, NQ], F16, name="cpT")
        k1hT = persist.tile([128, 2, NK], F16, name="k1hT")
        k2hT = persist.tile([128, 2, NK], F16, name="k2hT")
        vh = persist.tile([128, 32, 2, 128], F16, name="vh")
        u_store = persist.tile([128, 2, 16, 128], F16, name="u_store")

        acts = ctx.enter_context(tc.tile_pool(name="acts", bufs=3))
        ppsum = ctx.enter_context(tc.tile_pool(name="ppsum", bufs=1, space="PSUM"))
        spsum = ctx.enter_context(tc.tile_pool(name="spsum", bufs=2, space="PSUM"))
        ptpsum = ctx.enter_context(tc.tile_pool(name="ptpsum", bufs=1, space="PSUM"))
        upsum = ctx.enter_context(tc.tile_pool(name="upsum", bufs=1, space="PSUM"))
        opsum = ctx.enter_context(tc.tile_pool(name="opsum", bufs=1, space="PSUM"))
        ppool = ctx.enter_context(tc.tile_pool(name="ppool", bufs=3))
        ptpool = ctx.enter_context(tc.tile_pool(name="ptpool", bufs=2))
        small = ctx.enter_context(tc.tile_pool(name="small", bufs=3))
        opool = ctx.enter_context(tc.tile_pool(name="opool", bufs=2))

        def project_head(h):
            """Project k1hT/k2hT/vh/qpT/cpT for head h (slot h)."""
            hs = slice(h * 128, (h + 1) * 128)
            for blk in range(NK // 512):
                a_sb = acts.tile([128, 8, 512], F16, tag="act_in")
                nc.sync.dma_start(
                    out=a_sb,
                    in_=kT.rearrange("(c p) n -> p c n", p=128)[
                        :, :, blk * 512 : (blk + 1) * 512
                    ],
                )
                for wname, bname, dst in (
                    ("wk1", "bk1", k1hT),
                    ("wk2", "bk2", k2hT),
                ):
                    ps = ppsum.tile([128, 512], F32, tag="proj")
                    for e in range(8):
                        nc.tensor.matmul(
                            ps,
                            lhsT=w_sb[wname][:, e, hs],
                            rhs=a_sb[:, e, :],
                            start=(e == 0),
                            stop=(e == 7),
                        )
                    nc.vector.tensor_scalar_add(
                        dst[:, h, blk * 512 : (blk + 1) * 512],
                        ps,
                        b_sb[bname][:, h : h + 1],
                    )
            for blk in range(NK // 512):
                a_sb = acts.tile([128, 8, 512], F16, tag="act_in")
                nc.sync.dma_start(
                    out=a_sb,
                    in_=vT.rearrange("(c p) n -> p c n", p=128)[
                        :, :, blk * 512 : (blk + 1) * 512
                    ],
                )
                for tt in range(4):
                    ps = ppsum.tile([128, 512], F32, tag="proj", name="vps")[:, 0:128]
                    for e in range(8):
                        nc.tensor.matmul(
                            ps,
                            lhsT=a_sb[:, e, tt * 128 : (tt + 1) * 128],
                            rhs=w_sb["wv"][:, e, hs],
                            start=(e == 0),
                            stop=(e == 7),
                        )
                    nc.vector.tensor_copy(out=vh[:, blk * 4 + tt, h, :], in_=ps)
            for src, wname, bname, dst in (
                (qT, "wq", "bq", qpT),
                (cT, "wc", "bc", cpT),
            ):
                for blk in range(NQ // 512):
                    a_sb = acts.tile([128, 8, 512], F16, tag="act_in")
                    nc.sync.dma_start(
                        out=a_sb,
                        in_=src.rearrange("(c p) n -> p c n", p=128)[
                            :, :, blk * 512 : (blk + 1) * 512
                        ],
                    )
                    ps = ppsum.tile([128, 512], F32, tag="proj")
                    for e in range(8):
                        nc.tensor.matmul(
                            ps,
                            lhsT=w_sb[wname][:, e, hs],
                            rhs=a_sb[:, e, :],
                            start=(e == 0),
                            stop=(e == 7),
                        )
                    nc.vector.tensor_scalar_add(
                        dst[:, h, blk * 512 : (blk + 1) * 512],
                        ps,
                        b_sb[bname][:, h : h + 1],
                    )

        def attention_tile(t, h):
            """One nq-tile of attention for head h; final projection after h=1."""
            p_tiles = {}
            rparts = {}
            for m, src, khT in ((1, qpT, k1hT), (2, cpT, k2hT)):
                p_sb = ppool.tile([128, NK], F16, tag=f"p{m}")
                rp = small.tile([128, 4], F32, tag=f"rp{m}")
                lhsT = src[:, h, t * 128 : (t + 1) * 128]
                for qt in range(4):
                    ps = spsum.tile([128, 1024], F32, tag="s")
                    for j in range(2):
                        nc.tensor.matmul(
                            ps[:, j * 512 : (j + 1) * 512],
                            lhsT=lhsT,
                            rhs=khT[
                                :, h, qt * 1024 + j * 512 : qt * 1024 + (j + 1) * 512
                            ],
                            start=True,
                            stop=True,
                        )
                    nc.scalar.activation(
                        out=p_sb[:, qt * 1024 : (qt + 1) * 1024],
                        in_=ps,
                        func=ACTF.Exp,
                        scale=SCALE,
                        accum_out=rp[:, qt : qt + 1],
                    )
                p_tiles[m] = p_sb
                rparts[m] = rp

            betas = {}
            for m in (1, 2):
                r = small.tile([128, 1], F32, tag=f"r{m}")
                nc.vector.tensor_reduce(out=r, in_=rparts[m], axis=AX.X, op=ALU.add)
                rinv = small.tile([128, 1], F32, tag=f"rinv{m}")
                nc.vector.reciprocal(out=rinv, in_=r)
                beta = small.tile([128, 1], F32, tag=f"beta{m}")
                nc.vector.tensor_scalar_mul(beta, rinv, 0.5 * CNORM)
                betas[m] = beta

            pc = p_tiles[1]
            u_ps = upsum.tile([128, 128], F32, tag="u")
            for qt in range(4):
                qs = slice(qt * 1024, (qt + 1) * 1024)
                nc.vector.tensor_scalar_mul(
                    p_tiles[1][:, qs], p_tiles[1][:, qs], betas[1]
                )
                nc.vector.tensor_scalar_mul(
                    p_tiles[2][:, qs], p_tiles[2][:, qs], betas[2]
                )
                nc.vector.tensor_add(pc[:, qs], p_tiles[1][:, qs], p_tiles[2][:, qs])
                pt_ps = ptpsum.tile([128, 1024], F16, tag="pt")
                for c8 in range(8):
                    ch = qt * 8 + c8
                    nc.tensor.transpose(
                        pt_ps[:, c8 * 128 : (c8 + 1) * 128],
                        in_=pc[:, ch * 128 : (ch + 1) * 128],
                        identity=identity16,
                    )
                pt_sb = ptpool.tile([128, 1024], F16, tag="pt_sb")
                nc.vector.tensor_copy(out=pt_sb, in_=pt_ps)
                for c8 in range(8):
                    ch = qt * 8 + c8
                    nc.tensor.matmul(
                        u_ps,
                        lhsT=vh[:, ch, h, :],
                        rhs=pt_sb[:, c8 * 128 : (c8 + 1) * 128],
                        start=(ch == 0),
                        stop=(ch == 31),
                    )
            nc.vector.tensor_copy(out=u_store[:, h, t, :], in_=u_ps)

            if h == 1:
                o_sb = opool.tile([128, E], F32, tag="o_sb")
                for j in range(2):
                    o_ps = opsum.tile([128, 512], F32, tag="o")
                    for hh in range(2):
                        nc.tensor.matmul(
                            o_ps,
                            lhsT=u_store[:, hh, t, :],
                            rhs=wo_sb[:, hh, j * 512 : (j + 1) * 512],
                            start=(hh == 0),
                            stop=(hh == 1),
                        )
                    nc.vector.tensor_scalar_mul(
                        o_sb[:, j * 512 : (j + 1) * 512], o_ps, 1.0 / CNORM
                    )
                nc.sync.dma_start(out=outp[t * 128 : (t + 1) * 128, :], in_=o_sb)

        # software pipeline: project head 0, then attention(h) | project(h+1)
        project_head(0)
        for h in range(2):
            if h + 1 < 2:
                project_head(h + 1)
            for t in range(NQ // 128):
                attention_tile(t, h)

    nc.compile()
    return nc


def _get_nc():
    global _CACHED_NC
    if _CACHED_NC is None:
        _CACHED_NC = _build()
    return _CACHED_NC


def kernel(q, k, v, cond_feat, Wq, bq, Wc, bc, Wk2, bk2, Wv, bv, Wo, bo):
    from concourse.bass_utils import run_bass_kernel_spmd

    q = np.asarray(q, np.float32)
    k = np.asarray(k, np.float32)
    v = np.asarray(v, np.float32)
    cond_feat = np.asarray(cond_feat, np.float32)
    Wq, bq = np.asarray(Wq, np.float32), np.asarray(bq, np.float32)
    Wc, bc = np.asarray(Wc, np.float32), np.asarray(bc, np.float32)
    Wk2, bk2 = np.asarray(Wk2, np.float32), np.asarray(bk2, np.float32)
    Wv, bv = np.asarray(Wv, np.float32), np.asarray(bv, np.float32)
    Wo, bo = np.asarray(Wo, np.float32), np.asarray(bo, np.float32)

    f16 = lambda x: np.ascontiguousarray(x, dtype=np.float16)
    b2 = lambda x: np.ascontiguousarray(x.reshape(2, 128).T, dtype=np.float32)

    in_maps = []
    for core in range(8):
        b, g = core // 4, core % 4
        sl = slice(g * 256, (g + 1) * 256)
        sl2 = slice(E + g * 256, E + (g + 1) * 256)
        in_maps.append(
            {
                "qt": f16(q[b].T),
                "ct": f16(cond_feat[b].T),
                "kt": f16(k[b].T),
                "vt": f16(v[b].T),
                "wq": f16(Wq[:, sl]),
                "wc": f16(Wc[:, sl]),
                "wk1": f16(Wk2[:, sl]),
                "wk2": f16(Wk2[:, sl2]),
                "wv": f16(Wv[:, sl]),
                "wo": f16(Wo[sl, :]),
                "bq": b2(bq[sl]),
                "bc": b2(bc[sl]),
                "bk1": b2(bk2[sl]),
                "bk2": b2(bk2[sl2]),
            }
        )

    nc = _get_nc()
    res = run_bass_kernel_spmd(nc, in_maps, core_ids=list(range(8)))

    out = np.zeros((2, NQ, E), np.float32)
    for core in range(8):
        out[core // 4] += res.results[core]["outp"]
    out += (bv @ Wo + bo)[None, None, :]
    return out
